# revision 3
# baseline (speedup 1.0000x reference)
"""Trainium2 Bass kernel for nn_AttentionLayer_sigmoid (additive attention
sigmoid-gated sum-pool), data-parallel over batch on 8 NeuronCores.

Reference computation (per batch b):
    wq[l, h]  = sum_d mb[l, d] * W1[h, d]
    uh[h]     = sum_d input[d] * W2[h, d] + b2[h]
    s[l]      = sum_h v[h] * tanh(wq[l, h] + uh[h])
    align[l]  = sigmoid(s[l]) * mask[l]
    out[d]    = sum_l align[l] * mb[l, d]

Shapes: B=32, L=2048, D=H=768.  Sharding: batch across 8 cores (4 each).

Per-core device layout (all prepped on host):
    mbt  [4, 6, 128, 2048] bf16   mb transposed: [b, dc, p, l] = mb[b, l, dc*128+p]
    w1t  [6, 128, 768]     bf16   W1.T chunked:  [dc, p, h] = W1[h, dc*128+p]
    w2t  [6, 128, 768]     bf16   W2.T chunked
    int_ [6, 128, 4]       bf16   input.T chunked: [dc, p, b] = input[b, dc*128+p]
    b2c  [128, 6]          f32    [p, hc] = b2[hc*128+p]
    vc   [128, 6]          bf16   [p, hc] = v[hc*128+p]
    maskf[1, 8192]         bf16   [0, b*2048+l] = mask[b, l]
    ident[128, 128]        f32    identity (PE transpose operand)

Compute structure per core:
  - uh:   PE matmuls (W2.T chunks) -> psum -> ACT copy+bias(b2) -> uht [128, 24]
  - per (b, lt of 512):
      GEMM: 6x6 MMs -> wq psum [128, 512] (per h-chunk)
      tanh: ACT psum -> t bf16 [128, 512], bias = uht column (uh[h] for this b)
      vdot: 6 MMs (lhsT = v column) -> s psum [1, 512]
      sigmoid: ACT -> sig [1, 512]; mask: DVE mult -> align[b] [1, 2048]
  - per b: gpsimd partition_broadcast align -> [128, 2048];
      pooling: DVE tensor_tensor_reduce over mbt[b, dc] (contract free dim l)
      -> pool [128, 24] f32
  - PE transpose pool -> [24, 128] -> out [4, 768]
"""

import sys

sys.path.insert(0, "/opt/trn_rl_repo")

import numpy as np
import ml_dtypes

_B, _L, _D, _H = 32, 2048, 768, 768
_NCORES = 8
_BPC = _B // _NCORES  # batches per core = 4
_DC = _D // 128  # 6 d-chunks
_HC = _H // 128  # 6 h-chunks
_LTS = 512  # l-tile size
_LT = _L // _LTS  # 4 l-tiles per batch

_cache = {}


def _build():
    import concourse.bacc as bacc
    import concourse.tile as tile
    import concourse.mybir as mybir

    f32 = mybir.dt.float32
    bf16 = mybir.dt.bfloat16
    AF = mybir.ActivationFunctionType
    ALU = mybir.AluOpType

    nc = bacc.Bacc("TRN2", target_bir_lowering=False, debug=False)

    mbt = nc.dram_tensor("mbt", [_BPC, _DC, 128, _L], bf16, kind="ExternalInput")
    w1t = nc.dram_tensor("w1t", [_DC, 128, _H], bf16, kind="ExternalInput")
    w2t = nc.dram_tensor("w2t", [_DC, 128, _H], bf16, kind="ExternalInput")
    int_ = nc.dram_tensor("int_", [_DC, 128, _BPC], bf16, kind="ExternalInput")
    b2c = nc.dram_tensor("b2c", [128, _HC], f32, kind="ExternalInput")
    vc = nc.dram_tensor("vc", [128, _HC], bf16, kind="ExternalInput")
    maskf = nc.dram_tensor("maskf", [1, _BPC * _L], bf16, kind="ExternalInput")
    ident = nc.dram_tensor("ident", [128, 128], f32, kind="ExternalInput")
    out = nc.dram_tensor("out", [_BPC, _D], f32, kind="ExternalOutput")

    with tile.TileContext(nc) as tc:
        with (
            tc.tile_pool(name="const", bufs=1) as cpool,
            tc.tile_pool(name="mbt", bufs=1) as mpool,
            tc.tile_pool(name="t", bufs=2) as tpool,
            tc.tile_pool(name="sig", bufs=2) as sigpool,
            tc.tile_pool(name="scr", bufs=2) as scrpool,
            tc.tile_pool(name="wq", bufs=3, space="PSUM") as wqpool,
            tc.tile_pool(name="sps", bufs=2, space="PSUM") as spool,
            tc.tile_pool(name="mps", bufs=1, space="PSUM") as mpspool,
        ):
            # ---- constant loads ----
            w1t_sb = []
            w2t_sb = []
            int_sb = []
            for dc in range(_DC):
                t1 = cpool.tile([128, _H], bf16, tag=f"w1t{dc}")
                nc.sync.dma_start(t1[:], w1t[dc])
                w1t_sb.append(t1)
                t2 = cpool.tile([128, _H], bf16, tag=f"w2t{dc}")
                nc.sync.dma_start(t2[:], w2t[dc])
                w2t_sb.append(t2)
                t3 = cpool.tile([128, _BPC], bf16, tag=f"int{dc}")
                nc.sync.dma_start(t3[:], int_[dc])
                int_sb.append(t3)
            b2c_sb = cpool.tile([128, _HC], f32, tag="b2c")
            nc.sync.dma_start(b2c_sb[:], b2c[:])
            vc_sb = cpool.tile([128, _HC], bf16, tag="vc")
            nc.sync.dma_start(vc_sb[:], vc[:])
            maskf_sb = cpool.tile([1, _BPC * _L], bf16, tag="maskf")
            nc.sync.dma_start(maskf_sb[:], maskf[:])
            ident_sb = cpool.tile([128, 128], f32, tag="ident")
            nc.sync.dma_start(ident_sb[:], ident[:])

            mbt_sb = [[None] * _DC for _ in range(_BPC)]
            for b in range(_BPC):
                for dc in range(_DC):
                    mt = mpool.tile([128, _L], bf16, tag=f"mbt{b}_{dc}")
                    nc.sync.dma_start(mt[:], mbt[b, dc])
                    mbt_sb[b][dc] = mt

            align_sb = []
            for b in range(_BPC):
                al = cpool.tile([1, _L], bf16, tag=f"align{b}")
                align_sb.append(al)
            align_bc = cpool.tile([128, _L], bf16, tag="align_bc")
            pool_sb = cpool.tile([128, _BPC * _DC], f32, tag="pool")
            uht_sb = cpool.tile([128, _HC * _BPC], f32, tag="uht")
            outT_sb = cpool.tile([_BPC * _DC, 128], f32, tag="outT")

            # ---- uh = input @ W2.T + b2  -> uht [128, hc*4+b] ----
            uh_ps = mpspool.tile([128, _HC * _BPC], f32, tag="uh")
            for hc in range(_HC):
                for dc in range(_DC):
                    nc.tensor.matmul(
                        uh_ps[:, hc * _BPC : (hc + 1) * _BPC],
                        w2t_sb[dc][:, hc * 128 : (hc + 1) * 128],
                        int_sb[dc][:],
                        start=(dc == 0),
                        stop=(dc == _DC - 1),
                    )
            for hc in range(_HC):
                nc.scalar.activation(
                    uht_sb[:, hc * _BPC : (hc + 1) * _BPC],
                    uh_ps[:, hc * _BPC : (hc + 1) * _BPC],
                    AF.Identity,
                    bias=b2c_sb[:, hc : hc + 1],
                )

            # ---- main loop ----
            for b in range(_BPC):
                for lt in range(_LT):
                    l0 = lt * _LTS
                    t_tiles = []
                    for hc in range(_HC):
                        wq = wqpool.tile([128, _LTS], f32, tag="wq")
                        for dc in range(_DC):
                            nc.tensor.matmul(
                                wq[:],
                                w1t_sb[dc][:, hc * 128 : (hc + 1) * 128],
                                mbt_sb[b][dc][:, l0 : l0 + _LTS],
                                start=(dc == 0),
                                stop=(dc == _DC - 1),
                            )
                        tt = tpool.tile([128, _LTS], bf16, tag=f"t{hc}")
                        nc.scalar.activation(
                            tt[:],
                            wq[:],
                            AF.Tanh,
                            bias=uht_sb[:, hc * _BPC + b : hc * _BPC + b + 1],
                        )
                        t_tiles.append(tt)
                    s_ps = spool.tile([1, _LTS], f32, tag="s")
                    for hc in range(_HC):
                        nc.tensor.matmul(
                            s_ps[:],
                            vc_sb[:, hc : hc + 1],
                            t_tiles[hc][:],
                            start=(hc == 0),
                            stop=(hc == _HC - 1),
                        )
                    sig = sigpool.tile([1, _LTS], bf16, tag="sig")
                    nc.scalar.activation(sig[:], s_ps[:], AF.Sigmoid)
                    nc.vector.tensor_tensor(
                        align_sb[b][:, l0 : l0 + _LTS],
                        sig[:],
                        maskf_sb[:, b * _L + l0 : b * _L + l0 + _LTS],
                        op=ALU.mult,
                    )
                # pooling for batch b
                nc.gpsimd.partition_broadcast(align_bc[:], align_sb[b][:])
                for dc in range(_DC):
                    scr = scrpool.tile([128, _L], bf16, tag="scr")
                    nc.vector.scalar_tensor_tensor(
                        out=scr[:],
                        in0=mbt_sb[b][dc][:],
                        scalar=1.0,
                        in1=align_bc[:],
                        op0=ALU.mult,
                        op1=ALU.mult,
                        accum_out=pool_sb[:, b * _DC + dc : b * _DC + dc + 1],
                    )

            # ---- final transpose + store ----
            outT_ps = mpspool.tile([_BPC * _DC, 128], f32, tag="outT")
            nc.tensor.transpose(outT_ps[:], pool_sb[:], ident_sb[:])
            nc.vector.tensor_copy(outT_sb[:], outT_ps[:])
            nc.sync.dma_start(
                out[:].rearrange("b (c d) -> (b c) d", d=128), outT_sb[:]
            )

    nc.compile()
    return nc


def _prep_inputs(input, memory_bank, memory_mask, W1, W2, b2, v):
    bf16 = ml_dtypes.bfloat16
    W1T = np.ascontiguousarray(W1.T).reshape(_DC, 128, _H).astype(bf16)
    W2T = np.ascontiguousarray(W2.T).reshape(_DC, 128, _H).astype(bf16)
    b2c = np.ascontiguousarray(b2.reshape(_HC, 128).T).astype(np.float32)
    vc = np.ascontiguousarray(v.reshape(_HC, 128).T).astype(bf16)
    ident = np.eye(128, dtype=np.float32)

    in_maps = []
    for i in range(_NCORES):
        sl = slice(i * _BPC, (i + 1) * _BPC)
        mb = memory_bank[sl]  # [4, L, D] f32
        # mbt[b, dc, p, l] = mb[b, l, dc*128+p]
        mbt = (
            np.ascontiguousarray(mb.transpose(0, 2, 1))  # [4, D, L]
            .reshape(_BPC, _DC, 128, _L)
            .astype(bf16)
        )
        int_ = (
            np.ascontiguousarray(input[sl].T).reshape(_DC, 128, _BPC).astype(bf16)
        )
        maskf = (
            memory_mask[sl].astype(np.float32).reshape(1, _BPC * _L).astype(bf16)
        )
        in_maps.append(
            {
                "mbt": mbt,
                "w1t": W1T,
                "w2t": W2T,
                "int_": int_,
                "b2c": b2c,
                "vc": vc,
                "maskf": maskf,
                "ident": ident,
            }
        )
    return in_maps


def kernel(input, memory_bank, memory_mask, W1, W2, b2, v):
    from concourse.bass_utils import run_bass_kernel_spmd

    input = np.asarray(input, dtype=np.float32)
    memory_bank = np.asarray(memory_bank, dtype=np.float32)
    memory_mask_np = np.asarray(memory_mask)
    W1 = np.asarray(W1, dtype=np.float32)
    W2 = np.asarray(W2, dtype=np.float32)
    b2 = np.asarray(b2, dtype=np.float32)
    v = np.asarray(v, dtype=np.float32)

    if "nc" not in _cache:
        _cache["nc"] = _build()
    nc = _cache["nc"]

    in_maps = _prep_inputs(input, memory_bank, memory_mask_np, W1, W2, b2, v)
    trace = _cache.get("trace", False)
    res = run_bass_kernel_spmd(
        nc,
        in_maps,
        core_ids=list(range(_NCORES)),
        trace=trace,
        **_cache.get("run_kwargs", {}),
    )
    _cache["last_result"] = res
    _cache["exec_time_ns"] = getattr(res, "exec_time_ns", None)
    outs = [np.asarray(r["out"], dtype=np.float32) for r in res.results]
    return np.concatenate(outs, axis=0)


# revision 15
# speedup vs baseline: 1.3469x; 1.3469x over previous
"""Trainium2 Bass kernel for nn_AttentionLayer_sigmoid (additive attention
sigmoid-gated sum-pool), data-parallel over batch on 8 NeuronCores.

Reference computation (per batch b):
    wq[l, h]  = sum_d mb[l, d] * W1[h, d]
    uh[h]     = sum_d input[d] * W2[h, d] + b2[h]
    s[l]      = sum_h v[h] * tanh(wq[l, h] + uh[h])
    align[l]  = sigmoid(s[l]) * mask[l]
    out[d]    = sum_l align[l] * mb[l, d]

Shapes: B=32, L=2048, D=H=768.  Sharding: batch across 8 cores (4 each).

Per-core device layout (all prepped on host):
    mbt  [4, 6, 128, 2048] bf16   mb transposed: [b, dc, p, l] = mb[b, l, dc*128+p]
    w1t  [6, 128, 768]     bf16   W1.T chunked:  [dc, p, h] = W1[h, dc*128+p]
    w2t  [6, 128, 768]     bf16   W2.T chunked
    int_ [6, 128, 4]       bf16   input.T chunked: [dc, p, b] = input[b, dc*128+p]
    b2c  [128, 6]          f32    [p, hc] = b2[hc*128+p]
    vc   [128, 6]          bf16   [p, hc] = v[hc*128+p]
    maskf[1, 8192]         bf16   [0, b*2048+l] = mask[b, l]
    ident[128, 128]        f32    identity (PE transpose operand)

Compute structure per core:
  - uh:   PE matmuls (W2.T chunks) -> psum -> ACT copy+bias(b2) -> uht [128, 24]
  - per (b, lt of 512):
      GEMM: 6x6 MMs -> wq psum [128, 512] (per h-chunk)
      tanh: ACT psum -> t bf16 [128, 512], bias = uht column (uh[h] for this b)
      vdot: 6 MMs (lhsT = v column) -> s psum [1, 512]
      sigmoid: ACT -> sig [1, 512]; mask: DVE mult -> align[b] [1, 2048]
  - per b: gpsimd partition_broadcast align -> [128, 2048];
      pooling: DVE tensor_tensor_reduce over mbt[b, dc] (contract free dim l)
      -> pool [128, 24] f32
  - PE transpose pool -> [24, 128] -> out [4, 768]
"""

import sys

sys.path.insert(0, "/opt/trn_rl_repo")

import numpy as np
import ml_dtypes

_B, _L, _D, _H = 32, 2048, 768, 768
_NCORES = 8
_BPC = _B // _NCORES  # batches per core = 4
_DC = _D // 128  # 6 d-chunks
_HC = _H // 128  # 6 h-chunks
_LTS = 512  # l-tile size
_LT = _L // _LTS  # 4 l-tiles per batch

_cache = {}


def _build():
    import concourse.bacc as bacc
    import concourse.tile as tile
    import concourse.mybir as mybir

    f32 = mybir.dt.float32
    bf16 = mybir.dt.bfloat16
    AF = mybir.ActivationFunctionType
    ALU = mybir.AluOpType

    fp8 = mybir.dt.float8e4
    PM = mybir.MatmulPerfMode

    nc = bacc.Bacc("TRN2", target_bir_lowering=False, debug=False)

    mbt = nc.dram_tensor("mbt", [_BPC, _DC, 128, _L], bf16, kind="ExternalInput")
    # DoubleRow operands: contraction index = (partition p, slot i) over 256 d
    # per dd-chunk; d = dd*256 + i*128 + p.  W1 is pre-scaled by 64 on host
    # (fp8e4 subnormal range), compensated via tanh scale=1/64.
    mbtd = nc.dram_tensor("mbtd", [_BPC, _DC // 2, 128, 2, _L], fp8, kind="ExternalInput")
    w1td = nc.dram_tensor("w1td", [_DC // 2, 128, 2, _H], fp8, kind="ExternalInput")
    w2t = nc.dram_tensor("w2t", [_DC, 128, _H], bf16, kind="ExternalInput")
    int_ = nc.dram_tensor("int_", [_DC, 128, _BPC], bf16, kind="ExternalInput")
    b2c = nc.dram_tensor("b2c", [128, _HC], f32, kind="ExternalInput")
    vc = nc.dram_tensor("vc", [128, _HC], bf16, kind="ExternalInput")
    maskf = nc.dram_tensor("maskf", [1, _BPC * _L], bf16, kind="ExternalInput")
    ident = nc.dram_tensor("ident", [128, 128], f32, kind="ExternalInput")
    out = nc.dram_tensor("out", [_BPC, _D], f32, kind="ExternalOutput")

    with tile.TileContext(nc) as tc:
        with (
            tc.tile_pool(name="const", bufs=1) as cpool,
            tc.tile_pool(name="mbt", bufs=2) as mpool,
            tc.tile_pool(name="t", bufs=2) as tpool,
            tc.tile_pool(name="sig", bufs=2) as sigpool,
            tc.tile_pool(name="scr", bufs=2) as scrpool,
            tc.tile_pool(name="wq", bufs=2, space="PSUM") as wqpool,
            tc.tile_pool(name="sps", bufs=2, space="PSUM") as spool,
            tc.tile_pool(name="mps", bufs=1, space="PSUM") as mpspool,
        ):
            # ---- constant loads ----
            w1td_sb = []
            for dd in range(_DC // 2):
                t0 = cpool.tile([128, 2, _H], fp8, tag=f"w1td{dd}")
                nc.sync.dma_start(t0[:], w1td[dd])
                w1td_sb.append(t0)
            w2t_sb = []
            int_sb = []
            for dc in range(_DC):
                t2 = cpool.tile([128, _H], bf16, tag=f"w2t{dc}")
                nc.sync.dma_start(t2[:], w2t[dc])
                w2t_sb.append(t2)
                t3 = cpool.tile([128, _BPC], bf16, tag=f"int{dc}")
                nc.sync.dma_start(t3[:], int_[dc])
                int_sb.append(t3)
            b2c_sb = cpool.tile([128, _HC], f32, tag="b2c")
            nc.sync.dma_start(b2c_sb[:], b2c[:])
            vc_sb = cpool.tile([128, _HC], bf16, tag="vc")
            nc.sync.dma_start(vc_sb[:], vc[:])
            maskf_sb = cpool.tile([1, _BPC * _L], bf16, tag="maskf")
            nc.sync.dma_start(maskf_sb[:], maskf[:])
            ident_sb = cpool.tile([128, 128], f32, tag="ident")
            nc.sync.dma_start(ident_sb[:], ident[:])

            align_sb = []
            for b in range(_BPC):
                al = cpool.tile([1, _L], bf16, tag=f"align{b}")
                align_sb.append(al)
            align_bc = cpool.tile([128, _L], bf16, tag="align_bc")
            pool_sb = cpool.tile([128, _BPC * _DC], f32, tag="pool")
            uht_sb = cpool.tile([128, _HC * _BPC], f32, tag="uht")
            outT_sb = cpool.tile([_BPC * _DC, 128], f32, tag="outT")

            # ---- uh = input @ W2.T + b2  -> uht [128, hc*4+b] ----
            uh_ps = mpspool.tile([128, _HC * _BPC], f32, tag="uh")
            for hc in range(_HC):
                for dc in range(_DC):
                    nc.tensor.matmul(
                        uh_ps[:, hc * _BPC : (hc + 1) * _BPC],
                        w2t_sb[dc][:, hc * 128 : (hc + 1) * 128],
                        int_sb[dc][:],
                        start=(dc == 0),
                        stop=(dc == _DC - 1),
                    )
            for hc in range(_HC):
                nc.scalar.activation(
                    uht_sb[:, hc * _BPC : (hc + 1) * _BPC],
                    uh_ps[:, hc * _BPC : (hc + 1) * _BPC],
                    AF.Identity,
                    bias=b2c_sb[:, hc : hc + 1],
                )

            # ---- main loop ----
            for b in range(_BPC):
                # stream this batch's memory-bank tiles (double-buffered pool)
                mbtd_b = []
                for dd in range(_DC // 2):
                    md = mpool.tile([128, 2, _L], fp8, tag=f"mbtd{dd}")
                    nc.sync.dma_start(md[:], mbtd[b, dd])
                    mbtd_b.append(md)
                mbt_b = []
                for dc in range(_DC):
                    mt = mpool.tile([128, _L], bf16, tag=f"mbt{dc}")
                    nc.sync.dma_start(mt[:], mbt[b, dc])
                    mbt_b.append(mt)
                for ltp in range(_L // 1024):
                    t_tiles = []
                    for hc in range(_HC):
                        wq = wqpool.tile([128, 1024], f32, tag="wq")
                        for half in range(2):
                            l0 = ltp * 1024 + half * _LTS
                            for dd in range(_DC // 2):
                                nc.tensor.matmul(
                                    wq[:, half * _LTS : (half + 1) * _LTS],
                                    w1td_sb[dd][:, :, hc * 128 : (hc + 1) * 128],
                                    mbtd_b[dd][:, :, l0 : l0 + _LTS],
                                    start=(dd == 0),
                                    stop=(dd == _DC // 2 - 1),
                                    perf_mode=PM.DoubleRow,
                                )
                        tt = tpool.tile([128, 1024], bf16, tag=f"t{hc}")
                        nc.scalar.activation(
                            tt[:],
                            wq[:],
                            AF.Tanh,
                            bias=uht_sb[:, hc * _BPC + b : hc * _BPC + b + 1],
                            scale=1.0 / 64.0,
                        )
                        t_tiles.append(tt)
                    for half in range(2):
                        l0 = ltp * 1024 + half * _LTS
                        s_ps = spool.tile([1, _LTS], f32, tag="s")
                        for hc in range(_HC):
                            nc.tensor.matmul(
                                s_ps[:],
                                vc_sb[:, hc : hc + 1],
                                t_tiles[hc][:, half * _LTS : (half + 1) * _LTS],
                                start=(hc == 0),
                                stop=(hc == _HC - 1),
                            )
                        sig = sigpool.tile([1, _LTS], bf16, tag="sig")
                        nc.scalar.activation(sig[:], s_ps[:], AF.Sigmoid)
                        nc.vector.tensor_tensor(
                            align_sb[b][:, l0 : l0 + _LTS],
                            sig[:],
                            maskf_sb[:, b * _L + l0 : b * _L + l0 + _LTS],
                            op=ALU.mult,
                        )
                # pooling for batch b
                nc.gpsimd.partition_broadcast(align_bc[:], align_sb[b][:])
                for dc in range(_DC):
                    scr = scrpool.tile([128, _L], bf16, tag="scr")
                    nc.vector.scalar_tensor_tensor(
                        out=scr[:],
                        in0=mbt_b[dc][:],
                        scalar=1.0,
                        in1=align_bc[:],
                        op0=ALU.mult,
                        op1=ALU.mult,
                        accum_out=pool_sb[:, b * _DC + dc : b * _DC + dc + 1],
                    )

            # ---- final transpose + store ----
            outT_ps = mpspool.tile([_BPC * _DC, 128], f32, tag="outT")
            nc.tensor.transpose(outT_ps[:], pool_sb[:], ident_sb[:])
            nc.vector.tensor_copy(outT_sb[:], outT_ps[:])
            nc.sync.dma_start(
                out[:].rearrange("b (c d) -> (b c) d", d=128), outT_sb[:]
            )

    nc.compile()
    return nc


def _prep_inputs(input, memory_bank, memory_mask, W1, W2, b2, v):
    bf16 = ml_dtypes.bfloat16
    fp8 = ml_dtypes.float8_e4m3
    # W1 values (~U[-0.036, 0.036]) sit in fp8e4 subnormal range; pre-scale
    # by 64 and compensate with scale=1/64 inside the tanh activation.
    # DoubleRow layout: [dd, p, i, h] = 64 * W1[h, dd*256 + i*128 + p]
    W1Ts = (64.0 * W1.T).reshape(_DC // 2, 2, 128, _H)
    W1TD = np.ascontiguousarray(W1Ts.transpose(0, 2, 1, 3)).astype(fp8)
    W2T = np.ascontiguousarray(W2.T).reshape(_DC, 128, _H).astype(bf16)
    b2c = np.ascontiguousarray(b2.reshape(_HC, 128).T).astype(np.float32)
    vc = np.ascontiguousarray(v.reshape(_HC, 128).T).astype(bf16)
    ident = np.eye(128, dtype=np.float32)

    in_maps = []
    for i in range(_NCORES):
        sl = slice(i * _BPC, (i + 1) * _BPC)
        mb = memory_bank[sl]  # [4, L, D] f32
        mbT = np.ascontiguousarray(mb.transpose(0, 2, 1))  # [4, D, L]
        # mbt[b, dc, p, l] = mb[b, l, dc*128+p]
        mbt = mbT.reshape(_BPC, _DC, 128, _L).astype(bf16)
        # mbtd[b, dd, p, i, l] = mb[b, l, dd*256 + i*128 + p]
        mbtd = np.ascontiguousarray(
            mbT.reshape(_BPC, _DC // 2, 2, 128, _L).transpose(0, 1, 3, 2, 4)
        ).astype(fp8)
        int_ = (
            np.ascontiguousarray(input[sl].T).reshape(_DC, 128, _BPC).astype(bf16)
        )
        maskf = (
            memory_mask[sl].astype(np.float32).reshape(1, _BPC * _L).astype(bf16)
        )
        in_maps.append(
            {
                "mbt": mbt,
                "mbtd": mbtd,
                "w1td": W1TD,
                "w2t": W2T,
                "int_": int_,
                "b2c": b2c,
                "vc": vc,
                "maskf": maskf,
                "ident": ident,
            }
        )
    return in_maps


def kernel(input, memory_bank, memory_mask, W1, W2, b2, v):
    from concourse.bass_utils import run_bass_kernel_spmd

    input = np.asarray(input, dtype=np.float32)
    memory_bank = np.asarray(memory_bank, dtype=np.float32)
    memory_mask_np = np.asarray(memory_mask)
    W1 = np.asarray(W1, dtype=np.float32)
    W2 = np.asarray(W2, dtype=np.float32)
    b2 = np.asarray(b2, dtype=np.float32)
    v = np.asarray(v, dtype=np.float32)

    if "nc" not in _cache:
        _cache["nc"] = _build()
    nc = _cache["nc"]

    in_maps = _prep_inputs(input, memory_bank, memory_mask_np, W1, W2, b2, v)
    trace = _cache.get("trace", False)
    res = run_bass_kernel_spmd(
        nc,
        in_maps,
        core_ids=list(range(_NCORES)),
        trace=trace,
        **_cache.get("run_kwargs", {}),
    )
    _cache["last_result"] = res
    _cache["exec_time_ns"] = getattr(res, "exec_time_ns", None)
    outs = [np.asarray(r["out"], dtype=np.float32) for r in res.results]
    return np.concatenate(outs, axis=0)


# revision 24
# speedup vs baseline: 1.4720x; 1.0929x over previous
"""Trainium2 Bass kernel for nn_AttentionLayer_sigmoid (additive attention
sigmoid-gated sum-pool), data-parallel over batch on 8 NeuronCores.

Reference computation (per batch b):
    wq[l, h]  = sum_d mb[l, d] * W1[h, d]
    uh[h]     = sum_d input[d] * W2[h, d] + b2[h]
    s[l]      = sum_h v[h] * tanh(wq[l, h] + uh[h])
    align[l]  = sigmoid(s[l]) * mask[l]
    out[d]    = sum_l align[l] * mb[l, d]

Shapes: B=32, L=2048, D=H=768.  Sharding: batch across 8 cores (4 each).

Per-core device layout (all prepped on host):
    mbt  [4, 6, 128, 2048] bf16   mb transposed: [b, dc, p, l] = mb[b, l, dc*128+p]
    w1t  [6, 128, 768]     bf16   W1.T chunked:  [dc, p, h] = W1[h, dc*128+p]
    w2t  [6, 128, 768]     bf16   W2.T chunked
    int_ [6, 128, 4]       bf16   input.T chunked: [dc, p, b] = input[b, dc*128+p]
    b2c  [128, 6]          f32    [p, hc] = b2[hc*128+p]
    vc   [128, 6]          bf16   [p, hc] = v[hc*128+p]
    maskf[1, 8192]         bf16   [0, b*2048+l] = mask[b, l]
    ident[128, 128]        f32    identity (PE transpose operand)

Compute structure per core:
  - uh:   PE matmuls (W2.T chunks) -> psum -> ACT copy+bias(b2) -> uht [128, 24]
  - per (b, lt of 512):
      GEMM: 6x6 MMs -> wq psum [128, 512] (per h-chunk)
      tanh: ACT psum -> t bf16 [128, 512], bias = uht column (uh[h] for this b)
      vdot: 6 MMs (lhsT = v column) -> s psum [1, 512]
      sigmoid: ACT -> sig [1, 512]; mask: DVE mult -> align[b] [1, 2048]
  - per b: gpsimd partition_broadcast align -> [128, 2048];
      pooling: DVE tensor_tensor_reduce over mbt[b, dc] (contract free dim l)
      -> pool [128, 24] f32
  - PE transpose pool -> [24, 128] -> out [4, 768]
"""

import sys

sys.path.insert(0, "/opt/trn_rl_repo")

import numpy as np
import ml_dtypes

_B, _L, _D, _H = 32, 2048, 768, 768
_NCORES = 8
_BPC = _B // _NCORES  # batches per core = 4
_DC = _D // 128  # 6 d-chunks
_HC = _H // 128  # 6 h-chunks
_LTS = 512  # l-tile size
_LT = _L // _LTS  # 4 l-tiles per batch

_cache = {}


def _build():
    import concourse.bacc as bacc
    import concourse.tile as tile
    import concourse.mybir as mybir

    f32 = mybir.dt.float32
    bf16 = mybir.dt.bfloat16
    AF = mybir.ActivationFunctionType
    ALU = mybir.AluOpType

    fp8 = mybir.dt.float8e4
    PM = mybir.MatmulPerfMode

    nc = bacc.Bacc("TRN2", target_bir_lowering=False, debug=False)

    mbt = nc.dram_tensor("mbt", [_BPC, _DC, 128, _L], bf16, kind="ExternalInput")
    # DoubleRow operands: contraction index = (partition p, slot i) over 256 d
    # per dd-chunk; d = dd*256 + i*128 + p.  W1 is pre-scaled by 64 on host
    # (fp8e4 subnormal range), compensated via tanh scale=1/64.
    mbtd = nc.dram_tensor("mbtd", [_BPC, _DC // 2, 128, 2, _L], fp8, kind="ExternalInput")
    w1td = nc.dram_tensor("w1td", [_DC // 2, 128, 2, _H], fp8, kind="ExternalInput")
    w2t = nc.dram_tensor("w2t", [_DC, 128, _H], bf16, kind="ExternalInput")
    int_ = nc.dram_tensor("int_", [_DC, 128, _BPC], bf16, kind="ExternalInput")
    b2c = nc.dram_tensor("b2c", [128, _HC], f32, kind="ExternalInput")
    vcd = nc.dram_tensor("vcd", [128, 2, 16], fp8, kind="ExternalInput")
    maskf = nc.dram_tensor("maskf", [1, _BPC * _L], bf16, kind="ExternalInput")
    ident = nc.dram_tensor("ident", [128, 128], f32, kind="ExternalInput")
    out = nc.dram_tensor("out", [_BPC, _D], f32, kind="ExternalOutput")

    with tile.TileContext(nc) as tc:
        with (
            tc.tile_pool(name="const", bufs=1) as cpool,
            tc.tile_pool(name="mbt", bufs=2) as mpool,
            tc.tile_pool(name="t", bufs=2) as tpool,
            tc.tile_pool(name="sig", bufs=2) as sigpool,
            tc.tile_pool(name="scr", bufs=2) as scrpool,
            tc.tile_pool(name="wq", bufs=2, space="PSUM") as wqpool,
            tc.tile_pool(name="sps", bufs=2, space="PSUM") as spool,
            tc.tile_pool(name="mps", bufs=1, space="PSUM") as mpspool,
        ):
            # ---- constant loads ----
            w1td_sb = []
            for dd in range(_DC // 2):
                t0 = cpool.tile([128, 2, _H], fp8, tag=f"w1td{dd}")
                nc.sync.dma_start(t0[:], w1td[dd])
                w1td_sb.append(t0)
            w2t_sb = []
            int_sb = []
            for dc in range(_DC):
                t2 = cpool.tile([128, _H], bf16, tag=f"w2t{dc}")
                nc.sync.dma_start(t2[:], w2t[dc])
                w2t_sb.append(t2)
                t3 = cpool.tile([128, _BPC], bf16, tag=f"int{dc}")
                nc.sync.dma_start(t3[:], int_[dc])
                int_sb.append(t3)
            b2c_sb = cpool.tile([128, _HC], f32, tag="b2c")
            nc.sync.dma_start(b2c_sb[:], b2c[:])
            vcd_sb = cpool.tile([128, 2, 16], fp8, tag="vcd")
            nc.sync.dma_start(vcd_sb[:], vcd[:])
            maskf_sb = cpool.tile([1, _BPC * _L], bf16, tag="maskf")
            nc.sync.dma_start(maskf_sb[:], maskf[:])
            ident_sb = cpool.tile([128, 128], f32, tag="ident")
            nc.sync.dma_start(ident_sb[:], ident[:])

            align_sb = []
            for b in range(_BPC):
                al = cpool.tile([1, _L], bf16, tag=f"align{b}")
                align_sb.append(al)
            pool_sb = cpool.tile([128, _BPC * _DC], f32, tag="pool")
            uht_sb = cpool.tile([128, _HC * _BPC], f32, tag="uht")
            outT_sb = cpool.tile([_BPC * _DC, 128], f32, tag="outT")

            # ---- uh = input @ W2.T + b2  -> uht [128, hc*4+b] ----
            uh_ps = mpspool.tile([128, _HC * _BPC], f32, tag="uh")
            for hc in range(_HC):
                for dc in range(_DC):
                    nc.tensor.matmul(
                        uh_ps[:, hc * _BPC : (hc + 1) * _BPC],
                        w2t_sb[dc][:, hc * 128 : (hc + 1) * 128],
                        int_sb[dc][:],
                        start=(dc == 0),
                        stop=(dc == _DC - 1),
                    )
            for hc in range(_HC):
                nc.scalar.activation(
                    uht_sb[:, hc * _BPC : (hc + 1) * _BPC],
                    uh_ps[:, hc * _BPC : (hc + 1) * _BPC],
                    AF.Identity,
                    bias=b2c_sb[:, hc : hc + 1],
                )

            # ---- main loop ----
            for b in range(_BPC):
                # stream this batch's memory-bank tiles (double-buffered pool)
                mbtd_b = []
                for dd in range(_DC // 2):
                    md = mpool.tile([128, 2, _L], fp8, tag=f"mbtd{dd}")
                    nc.sync.dma_start(md[:], mbtd[b, dd])
                    mbtd_b.append(md)
                mbt_b = []
                for dc in range(_DC):
                    mt = mpool.tile([128, _L], bf16, tag=f"mbt{dc}")
                    nc.sync.dma_start(mt[:], mbt[b, dc])
                    mbt_b.append(mt)
                for ltp in range(_L // 1024):
                    t_pairs = []
                    for hp in range(_HC // 2):
                        tp = tpool.tile([128, 2, 1024], fp8, tag=f"tp{hp}")
                        t_pairs.append(tp)
                        for sub in range(2):
                            hc = hp * 2 + sub
                            wq = wqpool.tile([128, 1024], f32, tag="wq")
                            for dd in range(_DC // 2):
                                for half in range(2):
                                    l0 = ltp * 1024 + half * _LTS
                                    nc.tensor.matmul(
                                        wq[:, half * _LTS : (half + 1) * _LTS],
                                        w1td_sb[dd][:, :, hc * 128 : (hc + 1) * 128],
                                        mbtd_b[dd][:, :, l0 : l0 + _LTS],
                                        start=(dd == 0),
                                        stop=(dd == _DC // 2 - 1),
                                        perf_mode=PM.DoubleRow,
                                    )
                            # t stored fp8 (x64 pre-scale baked into v instead)
                            nc.scalar.activation(
                                tp[:, sub, :],
                                wq[:],
                                AF.Tanh,
                                bias=uht_sb[:, hc * _BPC + b : hc * _BPC + b + 1],
                                scale=1.0 / 64.0,
                            )
                    for half in range(2):
                        l0 = ltp * 1024 + half * _LTS
                        s_ps = spool.tile([1, _LTS], f32, tag="s")
                        for hp in range(_HC // 2):
                            nc.tensor.matmul(
                                s_ps[:],
                                vcd_sb[:, :, hp : hp + 1],
                                t_pairs[hp][:, :, half * _LTS : (half + 1) * _LTS],
                                start=(hp == 0),
                                stop=(hp == _HC // 2 - 1),
                                perf_mode=PM.DoubleRow,
                            )
                        sig = sigpool.tile([1, _LTS], bf16, tag="sig")
                        nc.scalar.activation(sig[:], s_ps[:], AF.Sigmoid, scale=1.0 / 64.0)
                        nc.vector.tensor_tensor(
                            align_sb[b][:, l0 : l0 + _LTS],
                            sig[:],
                            maskf_sb[:, b * _L + l0 : b * _L + l0 + _LTS],
                            op=ALU.mult,
                        )
                # pooling for batch b: weighted free-dim reduce of mbt tiles,
                # split across DVE and GPSIMD to balance engine load
                align_bc = scrpool.tile([128, _L], bf16, tag="align_bc")
                nc.gpsimd.partition_broadcast(align_bc[:], align_sb[b][:])
                for dc in range(_DC):
                    eng = nc.vector
                    scr = scrpool.tile([128, _L], bf16, tag="scr")
                    eng.scalar_tensor_tensor(
                        out=scr[:],
                        in0=mbt_b[dc][:],
                        scalar=1.0,
                        in1=align_bc[:],
                        op0=ALU.mult,
                        op1=ALU.mult,
                        accum_out=pool_sb[:, b * _DC + dc : b * _DC + dc + 1],
                    )

            # ---- final transpose + store ----
            outT_ps = mpspool.tile([_BPC * _DC, 128], f32, tag="outT")
            nc.tensor.transpose(outT_ps[:], pool_sb[:], ident_sb[:])
            nc.vector.tensor_copy(outT_sb[:], outT_ps[:])
            nc.sync.dma_start(
                out[:].rearrange("b (c d) -> (b c) d", d=128), outT_sb[:]
            )

    nc.compile()
    return nc


def _prep_inputs(input, memory_bank, memory_mask, W1, W2, b2, v):
    bf16 = ml_dtypes.bfloat16
    fp8 = ml_dtypes.float8_e4m3
    # W1 values (~U[-0.036, 0.036]) sit in fp8e4 subnormal range; pre-scale
    # by 64 and compensate with scale=1/64 inside the tanh activation.
    # DoubleRow layout: [dd, p, i, h] = 64 * W1[h, dd*256 + i*128 + p]
    W1Ts = (64.0 * W1.T).reshape(_DC // 2, 2, 128, _H)
    W1TD = np.ascontiguousarray(W1Ts.transpose(0, 2, 1, 3)).astype(fp8)
    W2T = np.ascontiguousarray(W2.T).reshape(_DC, 128, _H).astype(bf16)
    b2c = np.ascontiguousarray(b2.reshape(_HC, 128).T).astype(np.float32)
    # vcd[p, i, hp] = 64 * v[(2*hp+i)*128 + p]  (fp8 subnormal pre-scale)
    vcd = np.zeros((128, 2, 16), dtype=fp8)
    vcd[:, :, : _HC // 2] = (
        (64.0 * v).reshape(_HC // 2, 2, 128).transpose(2, 1, 0)
    ).astype(fp8)
    ident = np.eye(128, dtype=np.float32)

    in_maps = []
    for i in range(_NCORES):
        sl = slice(i * _BPC, (i + 1) * _BPC)
        mb = memory_bank[sl]  # [4, L, D] f32
        mbT = np.ascontiguousarray(mb.transpose(0, 2, 1))  # [4, D, L]
        # mbt[b, dc, p, l] = mb[b, l, dc*128+p]
        mbt = mbT.reshape(_BPC, _DC, 128, _L).astype(bf16)
        # mbtd[b, dd, p, i, l] = mb[b, l, dd*256 + i*128 + p]
        mbtd = np.ascontiguousarray(
            mbT.reshape(_BPC, _DC // 2, 2, 128, _L).transpose(0, 1, 3, 2, 4)
        ).astype(fp8)
        int_ = (
            np.ascontiguousarray(input[sl].T).reshape(_DC, 128, _BPC).astype(bf16)
        )
        maskf = (
            memory_mask[sl].astype(np.float32).reshape(1, _BPC * _L).astype(bf16)
        )
        in_maps.append(
            {
                "mbt": mbt,
                "mbtd": mbtd,
                "w1td": W1TD,
                "w2t": W2T,
                "int_": int_,
                "b2c": b2c,
                "vcd": vcd,
                "maskf": maskf,
                "ident": ident,
            }
        )
    return in_maps


def kernel(input, memory_bank, memory_mask, W1, W2, b2, v):
    from concourse.bass_utils import run_bass_kernel_spmd

    input = np.asarray(input, dtype=np.float32)
    memory_bank = np.asarray(memory_bank, dtype=np.float32)
    memory_mask_np = np.asarray(memory_mask)
    W1 = np.asarray(W1, dtype=np.float32)
    W2 = np.asarray(W2, dtype=np.float32)
    b2 = np.asarray(b2, dtype=np.float32)
    v = np.asarray(v, dtype=np.float32)

    if "nc" not in _cache:
        _cache["nc"] = _build()
    nc = _cache["nc"]

    in_maps = _prep_inputs(input, memory_bank, memory_mask_np, W1, W2, b2, v)
    trace = _cache.get("trace", False)
    res = run_bass_kernel_spmd(
        nc,
        in_maps,
        core_ids=list(range(_NCORES)),
        trace=trace,
        **_cache.get("run_kwargs", {}),
    )
    _cache["last_result"] = res
    _cache["exec_time_ns"] = getattr(res, "exec_time_ns", None)
    outs = [np.asarray(r["out"], dtype=np.float32) for r in res.results]
    return np.concatenate(outs, axis=0)
